# revision 1
# baseline (speedup 1.0000x reference)
"""Gemma3 sliding-window attention on 8 TRN2 NeuronCores via Bass/Tile.

Sharding: core c = b*4 + g  (b = batch, g = head-group):
  - q-heads {2g, 2g+1}, kv-head g, batch b  -> zero redundant projection work
  - column-shard wq/wk/wv, RoPE+RMSNorm local, blocked sliding-window
    attention, then a 4-rank bf16 AllGather of the attention outputs
    (concat over heads) per batch group, and a column-shard of wo.
All tensor-engine work in bf16 (f32 PSUM accumulation); softmax in f32->bf16.

Layout is fully transposed on-chip: Q^T/K^T are [head_dim, tokens] straight
out of the projection matmuls, V is [tokens, head_dim], scores are S^T
[keys, queries], attention output is O^T [head_dim, tokens] which directly
feeds the o-proj contraction. No transposes anywhere.
"""
import sys
import numpy as np

if "/opt/trn_rl_repo" not in sys.path:
    sys.path.insert(0, "/opt/trn_rl_repo")

from contextlib import ExitStack

import ml_dtypes
import concourse.bass as bass
import concourse.mybir as mybir
import concourse.tile as tile
from concourse import bacc
from concourse import bass_utils

BF16 = mybir.dt.bfloat16
F32 = mybir.dt.float32
NPBF16 = ml_dtypes.bfloat16

B, S, HID = 2, 2048, 2560
NH, NKV, HD = 8, 4, 256
SCALE = 256.0 ** -0.5
EPS = 1e-6
WIN = 1024
NCORES = 8
KT = HID // 128          # 20 k-tiles over hidden dim
QB = 512                 # query block (free dim of score matmuls)
NQB = S // QB            # 4 query blocks
TM = S // 128            # 16 token tiles of 128
NHID_LOC = 640           # per-core slice of o-proj output columns

REPLICA_GROUPS = [[0, 1, 2, 3], [4, 5, 6, 7]]


def _emit(nc, reps=1, stub_collective=False):
    AF = mybir.ActivationFunctionType

    xT = nc.dram_tensor("xT", [HID, S], BF16, kind="ExternalInput")
    wq = nc.dram_tensor("wq", [HID, 512], BF16, kind="ExternalInput")
    wk = nc.dram_tensor("wk", [HID, 256], BF16, kind="ExternalInput")
    wv = nc.dram_tensor("wv", [HID, 256], BF16, kind="ExternalInput")
    wo = nc.dram_tensor("wo", [NH * HD, NHID_LOC], BF16, kind="ExternalInput")
    cosT = nc.dram_tensor("cosT", [HD, S], BF16, kind="ExternalInput")
    rsinT = nc.dram_tensor("rsinT", [HD, S], BF16, kind="ExternalInput")
    qnw = nc.dram_tensor("qnw", [HD, 1], F32, kind="ExternalInput")
    knw = nc.dram_tensor("knw", [HD, 1], F32, kind="ExternalInput")
    maskb = nc.dram_tensor("maskb", [128, 1920], BF16, kind="ExternalInput")
    out = nc.dram_tensor("out", [S, NHID_LOC], F32, kind="ExternalOutput")

    xT_r = xT.rearrange("(t p) w -> p t w", p=128)        # [128, 20, 2048]
    wq_r = wq.rearrange("(t p) n -> p t n", p=128)        # [128, 20, 512]
    wk_r = wk.rearrange("(t p) n -> p t n", p=128)
    wv_r = wv.rearrange("(t p) n -> p t n", p=128)
    wo_r = wo.rearrange("(t p) n -> p t n", p=128)        # [128, 16, 640]
    cosT_r = cosT.rearrange("(d p) w -> p d w", p=128)    # [128, 2, 2048]
    rsinT_r = rsinT.rearrange("(d p) w -> p d w", p=128)
    qnw_r = qnw.rearrange("(d p) o -> p (d o)", p=128)    # [128, 2]
    knw_r = knw.rearrange("(d p) o -> p (d o)", p=128)

    with ExitStack() as ctx:
        tc = ctx.enter_context(tile.TileContext(nc))
        ec = ctx.enter_context
        constp = ec(tc.tile_pool(name="const", bufs=1))
        persist = ec(tc.tile_pool(name="persist", bufs=1))
        dram = ec(tc.tile_pool(name="dram", bufs=1, space="DRAM"))

        # ---- constants (live for the whole kernel) ----
        wq_s = constp.tile([128, KT, 512], BF16)
        wk_s = constp.tile([128, KT, 256], BF16)
        wv_s = constp.tile([128, KT, 256], BF16)
        mask_s = constp.tile([128, 1920], BF16)
        ones_s = constp.tile([128, 1], BF16)
        nc.vector.memset(ones_s[:], 1.0)

        # persistent activations
        qT_s = persist.tile([128, 2, 2, S], BF16)   # [p, head, hd-chunk, tok]
        kT_s = persist.tile([128, 2, S], BF16)      # [p, hd-chunk, tok]
        v_s = persist.tile([128, TM, 256], BF16)    # [p(tok), tok-tile, hd]
        wo_s = persist.tile([128, TM, NHID_LOC], BF16)

        for rep in range(reps):
            pending = []
            xt_tiles = {}
            xtp_rep = tc.tile_pool(name=f"xtr{rep}", bufs=2)
            xtp = xtp_rep.__enter__()

            def load_xt(tc4):
                t = xtp.tile([128, KT, QB], BF16, tag="xt")
                for g0 in range(0, KT, 5):
                    nc.sync.dma_start(
                        t[:, g0:g0 + 5, :],
                        xT_r[:, g0:g0 + 5, tc4 * QB:(tc4 + 1) * QB],
                    )
                xt_tiles[tc4] = t
            def emit_proj(half, tcs):
                # ================= phase 1: projections + RMSNorm + RoPE ==========
                p1 = ExitStack()
                scr = p1.enter_context(tc.tile_pool(name=f"scr{rep}_{half}", bufs=3))
                tiny = p1.enter_context(tc.tile_pool(name=f"tiny{rep}_{half}", bufs=2))
                ph1c = p1.enter_context(tc.tile_pool(name=f"ph1c{rep}_{half}", bufs=1))
                psq = p1.enter_context(tc.tile_pool(name=f"psq{rep}_{half}", bufs=2, space="PSUM"))
                psk = p1.enter_context(tc.tile_pool(name=f"psk{rep}_{half}", bufs=2, space="PSUM"))
                psv = p1.enter_context(tc.tile_pool(name=f"psv{rep}_{half}", bufs=2, space="PSUM"))
                psss = p1.enter_context(tc.tile_pool(name=f"psss{rep}_{half}", bufs=1, space="PSUM"))

                cos_s = ph1c.tile([128, 2, S], BF16)
                rsin_s = ph1c.tile([128, 2, S], BF16)
                qn_s = ph1c.tile([128, 2], F32)
                kn_s = ph1c.tile([128, 2], F32)

                def rms_rope(ps_pair, nw_s, dst, dst_h, t0):
                    """ps_pair: two [128, QB] f32 PSUM chunks of one head's ^T proj.
                    Normalize (RMS over the 256 partition dims), scale by per-dim
                    weight, apply RoPE, write bf16 into dst[:, (dst_h,) d, t0:t0+QB].
                    """
                    sqs = []
                    for d in range(2):
                        sq = scr.tile([128, QB], BF16, tag="sq")
                        nc.scalar.activation(sq[:], ps_pair[d][:], AF.Square)
                        sqs.append(sq)
                    ss = psss.tile([1, QB], F32, tag="ss")
                    nc.tensor.matmul(ss[:], ones_s[:], sqs[0][:], start=True, stop=False)
                    nc.tensor.matmul(ss[:], ones_s[:], sqs[1][:], start=False, stop=True)
                    tqs = []
                    for d in range(2):
                        tq = scr.tile([128, QB], BF16, tag="tq")
                        nc.vector.tensor_scalar(
                            tq[:], ps_pair[d][:], nw_s[:, d:d + 1], None,
                            mybir.AluOpType.mult,
                        )
                        tqs.append(tq)
                    ssn = tiny.tile([1, QB], F32, tag="ssn")
                    nc.scalar.activation(ssn[:], ss[:], AF.Copy, bias=EPS, scale=1.0 / HD)
                    rin = tiny.tile([1, QB], F32, tag="rin")
                    nc.vector.reciprocal(rin[:], ssn[:])
                    rr = tiny.tile([1, QB], BF16, tag="rr")
                    nc.scalar.activation(rr[:], rin[:], AF.Sqrt)
                    rrb = scr.tile([128, QB], BF16, tag="rrb")
                    nc.gpsimd.partition_broadcast(rrb[:], rr[:])
                    qrm = []
                    for d in range(2):
                        qr = scr.tile([128, QB], BF16, tag="qrm")
                        nc.vector.tensor_mul(qr[:], tqs[d][:], rrb[:])
                        qrm.append(qr)
                    for d in range(2):
                        a = scr.tile([128, QB], BF16, tag="ra")
                        b = scr.tile([128, QB], BF16, tag="rb")
                        nc.vector.tensor_mul(a[:], qrm[d][:], cos_s[:, d, t0:t0 + QB])
                        nc.vector.tensor_mul(b[:], qrm[1 - d][:], rsin_s[:, d, t0:t0 + QB])
                        if dst_h is None:
                            dslice = dst[:, d, t0:t0 + QB]
                        else:
                            dslice = dst[:, dst_h, d, t0:t0 + QB]
                        nc.vector.tensor_add(dslice, a[:], b[:])

                for tc4 in tcs:
                    if tc4 not in xt_tiles:
                        load_xt(tc4)
                    xt_s = xt_tiles.pop(tc4)
                    if tc4 == tcs[0]:
                        nc.sync.dma_start(qn_s[:], qnw_r[:])
                        nc.sync.dma_start(kn_s[:], knw_r[:])
                        nc.sync.dma_start(cos_s[:], cosT_r[:])
                        nc.sync.dma_start(rsin_s[:], rsinT_r[:])
                        if rep == 0 and half == 0:
                            nc.sync.dma_start(mask_s[:], maskb[:])
                    if True:
                        t0 = tc4 * QB
                        xs = xt_s[:, :, :]
                        # K^T for this token chunk
                        kps = []
                        for d in range(2):
                            pk = psk.tile([128, QB], F32, tag="pk")
                            for kt in range(KT):
                                nc.tensor.matmul(
                                    pk[:], wk_s[:, kt, d * 128:(d + 1) * 128],
                                    xs[:, kt, :], start=(kt == 0), stop=(kt == KT - 1),
                                )
                            kps.append(pk)
                        rms_rope(kps, kn_s, kT_s, None, t0)
                        # Q^T per head
                        for h in range(2):
                            qps = []
                            for d in range(2):
                                pq = psq.tile([128, QB], F32, tag="pq")
                                c = 2 * h + d
                                for kt in range(KT):
                                    nc.tensor.matmul(
                                        pq[:], wq_s[:, kt, c * 128:(c + 1) * 128],
                                        xs[:, kt, :], start=(kt == 0), stop=(kt == KT - 1),
                                    )
                                qps.append(pq)
                            rms_rope(qps, qn_s, qT_s, h, t0)
                        # V (natural layout) for the 4 token tiles in this chunk
                        for mm in range(4):
                            m = (t0 // 128) + mm
                            pv = psv.tile([128, 256], F32, tag="pv")
                            for kt in range(KT):
                                nc.tensor.matmul(
                                    pv[:], xs[:, kt, mm * 128:(mm + 1) * 128],
                                    wv_s[:, kt, :], start=(kt == 0), stop=(kt == KT - 1),
                                )
                            nc.vector.tensor_copy(v_s[:, m, :], pv[:])

                p1.close()

            def emit_att(half, qbs):
                # ========== phases 2+3 per query block: attn -> AllGather -> o-proj
                p2 = ExitStack()
                wop = p2.enter_context(tc.tile_pool(name=f"wop{rep}_{half}", bufs=1))
                esb = p2.enter_context(tc.tile_pool(name=f"esb{rep}_{half}", bufs=6))
                scr2 = p2.enter_context(tc.tile_pool(name=f"scr2{rep}_{half}", bufs=2))
                attp = p2.enter_context(tc.tile_pool(name=f"attp{rep}_{half}", bufs=2))
                agp = p2.enter_context(tc.tile_pool(name=f"agp{rep}_{half}", bufs=1))
                outpp = p2.enter_context(tc.tile_pool(name=f"outp{rep}_{half}", bufs=2))
                pss = p2.enter_context(tc.tile_pool(name=f"pss{rep}_{half}", bufs=3, space="PSUM"))
                pso = p2.enter_context(tc.tile_pool(name=f"pso{rep}_{half}", bufs=1, space="PSUM"))
                psse = p2.enter_context(tc.tile_pool(name=f"psse{rep}_{half}", bufs=1, space="PSUM"))
                psop = p2.enter_context(tc.tile_pool(name=f"psop{rep}_{half}", bufs=1, space="PSUM"))

                if half == 0 and rep == 0:
                    nc.sync.dma_start(wo_s[:], wo_r[:])

                def drain_one():
                    drain_oproj(limit=1)

                def drain_oproj(limit=99):
                    done = 0
                    while pending and done < limit:
                        done += 1
                        qb, agout = pending.pop(0)
                        ag_s = agp.tile([128, TM, QB], BF16, tag="ag")
                        agout_r2 = agout.rearrange("(t p) w -> p t w", p=128)
                        for mm in range(4):
                            nc.sync.dma_start(
                                ag_s[:, :, mm * 128:(mm + 1) * 128],
                                agout_r2[:, :, mm * 128:(mm + 1) * 128],
                            )
                        # o-proj for this block's 4 token tiles
                        for mm in range(4):
                            m = qb * 4 + mm
                            po = psop.tile([128, NHID_LOC], F32, tag="po")
                            for kt2 in range(TM):
                                lhs = ag_s[:, kt2, mm * 128:(mm + 1) * 128]
                                nc.tensor.matmul(
                                    po[:, 0:512], lhs, wo_s[:, kt2, 0:512],
                                    start=(kt2 == 0), stop=(kt2 == TM - 1),
                                )
                                nc.tensor.matmul(
                                    po[:, 512:NHID_LOC], lhs, wo_s[:, kt2, 512:NHID_LOC],
                                    start=(kt2 == 0), stop=(kt2 == TM - 1),
                                )
                            ot = outpp.tile([128, NHID_LOC], F32, tag="ot")
                            nc.vector.tensor_copy(ot[:, 0:512], po[:, 0:512])
                            nc.vector.tensor_copy(ot[:, 512:NHID_LOC], po[:, 512:NHID_LOC])
                            nc.sync.dma_start(out[m * 128:(m + 1) * 128, :], ot[:])

                for qb in qbs:
                    q0 = qb * QB
                    kt_lo = max(0, (q0 - WIN) // 128)
                    kt_hi = (q0 + QB - 1) // 128
                    att = attp.tile([128, 4, QB], BF16, tag="att")
                    for h in range(2):
                        o_ps = pso.tile([128, 2, QB], F32, tag="o")
                        se_ps = psse.tile([1, QB], F32, tag="se")
                        kts = list(range(kt_lo, kt_hi + 1))
                        # valid q-column range per k-tile: edge tiles are narrowed
                        # (outside columns are fully masked -> skip their compute)
                        rng = {}
                        for kt in kts:
                            d_off = q0 - kt * 128
                            lo_q = max(0, -d_off)
                            hi_q = min(QB, WIN + 128 - d_off) if d_off >= 640 else QB
                            rng[kt] = (lo_q, hi_q)
                        # PV accumulation order: a full-width tile first (so every
                        # psum column's first write has start=True), then the rest.
                        full0 = next(kt for kt in kts if rng[kt] == (0, QB))
                        pv_order = [full0] + [kt for kt in kts if kt != full0]
                        epipe = {}
                        emitted = [0]

                        se_emitted = [0]

                        def emit_pv(kt):
                            first, last = kt == pv_order[0], kt == pv_order[-1]
                            lo_q, hi_q = rng[kt]
                            e = epipe.pop(kt)
                            for d in range(2):
                                nc.tensor.matmul(
                                    o_ps[:, d, lo_q:hi_q],
                                    v_s[:, kt, d * 128:(d + 1) * 128],
                                    e[:, lo_q:hi_q], start=first, stop=last,
                                )

                        def drain_se(upto):
                            while se_emitted[0] < len(pv_order):
                                kt = pv_order[se_emitted[0]]
                                if kt not in epipe or kts.index(kt) > upto:
                                    break
                                lo_q, hi_q = rng[kt]
                                nc.tensor.matmul(
                                    se_ps[:, lo_q:hi_q], ones_s[:], epipe[kt][:, lo_q:hi_q],
                                    start=kt == pv_order[0], stop=kt == pv_order[-1],
                                )
                                se_emitted[0] += 1

                        def drain_pv(upto):
                            # emit pending PVs in pv_order whose exp tile exists
                            # and whose scores stage is >= 2 iterations old
                            while emitted[0] < len(pv_order):
                                kt = pv_order[emitted[0]]
                                if kt not in epipe or kts.index(kt) > upto:
                                    break
                                emit_pv(kt)
                                emitted[0] += 1

                        for i, kt in enumerate(kts):
                            k0 = kt * 128
                            d_off = q0 - k0
                            lo_q, hi_q = rng[kt]
                            s_ps = pss.tile([128, QB], F32, tag="s")
                            for d in range(2):
                                nc.tensor.matmul(
                                    s_ps[:, lo_q:hi_q], kT_s[:, d, k0:k0 + 128],
                                    qT_s[:, h, d, q0 + lo_q:q0 + hi_q],
                                    start=(d == 0), stop=(d == 1),
                                )
                            e_s = esb.tile([128, QB], BF16, tag="e")
                            nc.scalar.activation(e_s[:, lo_q:hi_q], s_ps[:, lo_q:hi_q],
                                                 AF.Exp)
                            if not (128 <= d_off <= 512):
                                nc.vector.tensor_mul(
                                    e_s[:, lo_q:hi_q], e_s[:, lo_q:hi_q],
                                    mask_s[:, 384 + d_off + lo_q:384 + d_off + hi_q],
                                )
                            epipe[kt] = e_s
                            drain_se(i)
                            drain_pv(i - 2)
                        drain_se(len(kts) - 1)
                        drain_pv(len(kts) - 1)
                        rc = scr2.tile([1, QB], F32, tag="rc")
                        nc.vector.reciprocal(rc[:], se_ps[:])
                        rcb = scr2.tile([128, QB], F32, tag="rcb")
                        nc.gpsimd.partition_broadcast(rcb[:], rc[:])
                        oc = scr2.tile([128, 2, QB], F32, tag="oc")
                        for d in range(2):
                            nc.vector.tensor_copy(oc[:, d, :], o_ps[:, d, :])
                        for d in range(2):
                            nc.vector.tensor_mul(att[:, 2 * h + d, :], oc[:, d, :], rcb[:])
                    # AllGather this block's attention outputs across the
                    # batch group; the gathered result is consumed by o-proj
                    # one half-pass later (drain_oproj), giving the collective
                    # queue a projection phase of runway.
                    agin = dram.tile([512, QB], BF16, tag=f"agin{rep}_{qb}")
                    agout = dram.tile([NH * HD, QB], BF16, tag=f"agout{rep}_{qb}")
                    agin_r = agin.rearrange("(c p) w -> c p w", p=128)
                    for c in range(4):
                        nc.sync.dma_start(agin_r[c], att[:, c, :])
                    if stub_collective:
                        agout_r = agout.rearrange("(r c p) w -> r c p w",
                                                  p=128, c=4)
                        for rr_i in range(4):
                            for cc in range(4):
                                nc.sync.dma_start(agout_r[rr_i, cc], att[:, cc, :])
                    else:
                        nc.gpsimd.collective_compute(
                            "AllGather",
                            mybir.AluOpType.bypass,
                            replica_groups=REPLICA_GROUPS,
                            ins=[agin[:]],
                            outs=[agout[:]],
                        )
                    pending.append((qb, agout))
                    if half == 1 and qb == qbs[0]:
                        # drain blocks 0/1 here: their gathers are long done,
                        # and this o-proj compute gives qb2's gather runway
                        while len(pending) > 1:
                            drain_one()
                    if half == 1 and qb == qbs[-1]:
                        drain_oproj()
                p2.close()

            if rep == 0:
                t0c = xtp.tile([128, KT, QB], BF16, tag="xt")
                for a, b in ((0, 2), (2, 5), (5, 10), (10, 15), (15, 20)):
                    nc.sync.dma_start(t0c[:, a:b, :], xT_r[:, a:b, 0:QB])
                    nc.sync.dma_start(wq_s[:, a:b, :], wq_r[:, a:b, :])
                    nc.sync.dma_start(wk_s[:, a:b, :], wk_r[:, a:b, :])
                    nc.sync.dma_start(wv_s[:, a:b, :], wv_r[:, a:b, :])
                xt_tiles[0] = t0c
            emit_proj(0, [0, 1])
            load_xt(2)
            emit_att(0, [0, 1])
            emit_proj(1, [2, 3])
            emit_att(1, [2, 3])
            xtp_rep.__exit__(None, None, None)


    nc.compile()
    return nc


_NC = {}


def _build(reps=1):
    if reps not in _NC:
        _NC[reps] = _emit(
            bacc.Bacc("TRN2", target_bir_lowering=False, debug=False,
                      num_devices=NCORES),
            reps=reps,
        )
    return _NC[reps]


def _host_prep(hidden_states, cos, sin, wq, wk, wv, wo, q_norm_w, k_norm_w):
    """Build the 8 per-core input maps (numpy, bf16 where device expects bf16)."""
    f32 = np.float32
    qn = ((1.0 + q_norm_w.astype(f32)) * SCALE).reshape(HD, 1)
    kn = (1.0 + k_norm_w.astype(f32)).reshape(HD, 1)
    # rsin: [-sin_firsthalf, +sin_secondhalf] so rope = q*cos + q[swap]*rsin
    # mask band: maskb[kk, y] = 1 iff 0 <= (y-384) - kk < WIN
    kk = np.arange(128)[:, None]
    y = np.arange(1920)[None, :]
    maskb = ((y - 384 - kk >= 0) & (y - 384 - kk < WIN)).astype(NPBF16)

    in_maps = []
    for c in range(NCORES):
        b, g = divmod(c, 4)
        sin_b = sin[b].astype(f32)
        rsin = np.concatenate([-sin_b[:, :128], sin_b[:, 128:]], axis=1)
        in_maps.append({
            "xT": np.ascontiguousarray(hidden_states[b].T).astype(NPBF16),
            "wq": np.ascontiguousarray(
                wq[:, 2 * g * HD:(2 * g + 2) * HD]).astype(NPBF16),
            "wk": np.ascontiguousarray(wk[:, g * HD:(g + 1) * HD]).astype(NPBF16),
            "wv": np.ascontiguousarray(wv[:, g * HD:(g + 1) * HD]).astype(NPBF16),
            "wo": np.ascontiguousarray(
                wo[:, g * NHID_LOC:(g + 1) * NHID_LOC]).astype(NPBF16),
            "cosT": np.ascontiguousarray(cos[b].T).astype(NPBF16),
            "rsinT": np.ascontiguousarray(rsin.T).astype(NPBF16),
            "qnw": qn,
            "knw": kn,
            "maskb": maskb,
        })
    return in_maps


class _Runner:
    """Compile the Bass module to a reusable 8-device PJRT executable
    (mirrors bass2jax.run_bass_via_pjrt but keeps the jitted fn for
    repeated steady-state invocation)."""

    def __init__(self, nc):
        import jax
        from jax.sharding import Mesh, PartitionSpec
        try:
            from jax import shard_map as _sm
            shard_map = _sm.shard_map if hasattr(_sm, "shard_map") else _sm
        except Exception:
            from jax.experimental.shard_map import shard_map
        from concourse import bass2jax
        from concourse.bass2jax import _bass_exec_p

        bass2jax.install_neuronx_cc_hook()
        self.jax = jax
        self.nc = nc
        part_name = (nc.partition_id_tensor.name
                     if nc.partition_id_tensor else None)
        in_names, out_names, out_avals = [], [], []
        for alloc in nc.m.functions[0].allocations:
            if not isinstance(alloc, mybir.MemoryLocationSet):
                continue
            name = alloc.memorylocations[0].name
            if alloc.kind == "ExternalInput":
                if name != part_name:
                    in_names.append(name)
            elif alloc.kind == "ExternalOutput":
                out_names.append(name)
                out_avals.append(jax.core.ShapedArray(
                    tuple(alloc.tensor_shape), mybir.dt.np(alloc.dtype)))
        self.in_names, self.out_names, self.out_avals = in_names, out_names, out_avals
        all_names = list(in_names) + list(out_names)
        if part_name is not None:
            all_names.append(part_name)

        def _body(*args):
            operands = list(args)
            if part_name is not None:
                operands.append(bass2jax.partition_id_tensor())
            outs = _bass_exec_p.bind(
                *operands,
                out_avals=tuple(out_avals),
                in_names=tuple(all_names),
                out_names=tuple(out_names),
                lowering_input_output_aliases=(),
                sim_require_finite=True,
                sim_require_nnan=True,
                nc=nc,
            )
            return tuple(outs)

        devices = jax.devices()[:NCORES]
        self.mesh = Mesh(np.asarray(devices), ("core",))
        n_args = len(in_names) + len(out_names)
        self.fn = jax.jit(
            shard_map(
                _body, mesh=self.mesh,
                in_specs=(PartitionSpec("core"),) * n_args,
                out_specs=(PartitionSpec("core"),) * len(out_names),
                check_vma=False,
            ),
            keep_unused=True,
        )
        self.sharding = jax.sharding.NamedSharding(
            self.mesh, PartitionSpec("core"))
        self.zeros = [
            jax.device_put(
                np.zeros((NCORES * a.shape[0], *a.shape[1:]), a.dtype),
                self.sharding)
            for a in out_avals
        ]

    def put(self, in_maps):
        concat = [
            np.concatenate([np.asarray(in_maps[c][n]) for c in range(NCORES)],
                           axis=0)
            for n in self.in_names
        ]
        return [self.jax.device_put(a, self.sharding) for a in concat]

    def run(self, in_dev):
        outs = self.fn(*in_dev, *self.zeros)
        return [o.block_until_ready() for o in outs]

    def results(self, outs):
        per_core = []
        for c in range(NCORES):
            m = {}
            for i, n in enumerate(self.out_names):
                a = self.out_avals[i]
                m[n] = np.asarray(outs[i]).reshape(NCORES, *a.shape)[c]
            per_core.append(m)
        return per_core


_RUNNER = None


def _get_runner():
    global _RUNNER
    if _RUNNER is None:
        _RUNNER = _Runner(_build())
    return _RUNNER


def kernel(hidden_states, cos, sin, wq, wk, wv, wo, q_norm_w, k_norm_w):
    global _RUNNER
    in_maps = _host_prep(hidden_states, cos, sin, wq, wk, wv, wo,
                         q_norm_w, k_norm_w)
    # The axon tunnel fails transiently (~1/3 of runs: mesh desync / exec-unit
    # unrecoverable). Retry the dispatch; on repeat failure rebuild the runner.
    last = None
    for attempt in range(4):
        try:
            r = _get_runner()
            res = r.results(r.run(r.put(in_maps)))
            break
        except Exception as e:  # transient axon/NRT dispatch failures
            last = e
            _RUNNER = None
    else:
        raise last
    out = np.empty((B, S, HID), np.float32)
    for b in range(B):
        out[b] = np.concatenate(
            [res[b * 4 + g]["out"] for g in range(4)], axis=1
        )
    return out



# revision 3
# speedup vs baseline: 1.4793x; 1.4793x over previous
"""Gemma3 sliding-window attention on 8 TRN2 NeuronCores via Bass/Tile.

Sharding: core c = b*4 + g  (b = batch, g = head-group):
  - q-heads {2g, 2g+1}, kv-head g, batch b  -> zero redundant projection work
  - column-shard wq/wk/wv, RoPE+RMSNorm local, blocked sliding-window
    attention, then a 4-rank bf16 AllGather of the attention outputs
    (concat over heads) per batch group, and a column-shard of wo.
All tensor-engine work in bf16 (f32 PSUM accumulation); softmax in f32->bf16.

Layout is fully transposed on-chip: Q^T/K^T are [head_dim, tokens] straight
out of the projection matmuls, V is [tokens, head_dim], scores are S^T
[keys, queries], attention output is O^T [head_dim, tokens] which directly
feeds the o-proj contraction. No transposes anywhere.

Pipelining: the o-proj for each query block is deferred by a full phase after
its AllGather, and the last two blocks of each rep drain during the NEXT
rep's projection phases, so the PE never waits on a collective. Softmax
denominators and RMS sums are computed with a ones[128,128] matmul so the
reciprocal runs as a full-width [128,512] DVE op (the [1,512] form is
lane-serial and ~8x slower).
"""
import sys
import numpy as np

if "/opt/trn_rl_repo" not in sys.path:
    sys.path.insert(0, "/opt/trn_rl_repo")

from contextlib import ExitStack

import ml_dtypes
import concourse.bass as bass
import concourse.mybir as mybir
import concourse.tile as tile
from concourse import bacc
from concourse import bass_utils

BF16 = mybir.dt.bfloat16
F32 = mybir.dt.float32
NPBF16 = ml_dtypes.bfloat16

B, S, HID = 2, 2048, 2560
NH, NKV, HD = 8, 4, 256
SCALE = 256.0 ** -0.5
WIN = 1024
NCORES = 8
KT = HID // 128          # 20 k-tiles over hidden dim
QB = 512                 # query block (free dim of score matmuls)
NQB = S // QB            # 4 query blocks
TM = S // 128            # 16 token tiles of 128
NHID_LOC = 640           # per-core slice of o-proj output columns

REPLICA_GROUPS = [[0, 1, 2, 3], [4, 5, 6, 7]]


def _emit(nc, reps=1, stub_collective=False):
    AF = mybir.ActivationFunctionType
    ALU = mybir.AluOpType

    xT = nc.dram_tensor("xT", [HID, S], BF16, kind="ExternalInput")
    wq = nc.dram_tensor("wq", [HID, 512], BF16, kind="ExternalInput")
    wk = nc.dram_tensor("wk", [HID, 256], BF16, kind="ExternalInput")
    wv = nc.dram_tensor("wv", [HID, 256], BF16, kind="ExternalInput")
    wo = nc.dram_tensor("wo", [NH * HD, NHID_LOC], BF16, kind="ExternalInput")
    cosT = nc.dram_tensor("cosT", [HD, S], BF16, kind="ExternalInput")
    rsinT = nc.dram_tensor("rsinT", [HD, S], BF16, kind="ExternalInput")
    qnw = nc.dram_tensor("qnw", [HD, 1], F32, kind="ExternalInput")
    knw = nc.dram_tensor("knw", [HD, 1], F32, kind="ExternalInput")
    maskb = nc.dram_tensor("maskb", [128, 1920], BF16, kind="ExternalInput")
    out = nc.dram_tensor("out", [S, NHID_LOC], BF16, kind="ExternalOutput")

    xT_r = xT.rearrange("(t p) w -> p t w", p=128)        # [128, 20, 2048]
    wq_r = wq.rearrange("(t p) n -> p t n", p=128)        # [128, 20, 512]
    wk_r = wk.rearrange("(t p) n -> p t n", p=128)
    wv_r = wv.rearrange("(t p) n -> p t n", p=128)
    wo_r = wo.rearrange("(t p) n -> p t n", p=128)        # [128, 16, 640]
    cosT_r = cosT.rearrange("(d p) w -> p d w", p=128)    # [128, 2, 2048]
    rsinT_r = rsinT.rearrange("(d p) w -> p d w", p=128)
    qnw_r = qnw.rearrange("(d p) o -> p (d o)", p=128)    # [128, 2]
    knw_r = knw.rearrange("(d p) o -> p (d o)", p=128)

    with ExitStack() as ctx:
        tc = ctx.enter_context(tile.TileContext(nc))
        ec = ctx.enter_context
        constp = ec(tc.tile_pool(name="const", bufs=1))
        persist = ec(tc.tile_pool(name="persist", bufs=1))
        dram = ec(tc.tile_pool(name="dram", bufs=1, space="DRAM"))
        # PSUM: 8 banks total.  b: 5 rotating [128,512] matmul targets;
        # r: sum-of-(squares|exps) broadcast rows; acc: shared 2-bank slot
        # for the attention O accumulator and the o-proj accumulator.
        psB = ec(tc.tile_pool(name="psB", bufs=5, space="PSUM"))
        psR = ec(tc.tile_pool(name="psR", bufs=1, space="PSUM"))
        psA = ec(tc.tile_pool(name="psA", bufs=1, space="PSUM"))
        xtp = ec(tc.tile_pool(name="xtp", bufs=2))
        scr = ec(tc.tile_pool(name="scr", bufs=2))
        esb = ec(tc.tile_pool(name="esb", bufs=5))
        attp = ec(tc.tile_pool(name="attp", bufs=2))
        agp = ec(tc.tile_pool(name="agp", bufs=1))
        outp = ec(tc.tile_pool(name="outp", bufs=2))

        # ---- constants (live for the whole kernel) ----
        wq_s = constp.tile([128, KT, 512], BF16)
        wk_s = constp.tile([128, KT, 256], BF16)
        wv_s = constp.tile([128, KT, 256], BF16)
        wo_s = constp.tile([128, TM, NHID_LOC], BF16)
        cos_s = constp.tile([128, 2, S], BF16)
        rsin_s = constp.tile([128, 2, S], BF16)
        qn_s = constp.tile([128, 2], F32)
        kn_s = constp.tile([128, 2], F32)
        mask_s = constp.tile([128, 1920], BF16)
        onesm = constp.tile([128, 128], BF16)
        nc.vector.memset(onesm[:], 1.0)

        # persistent activations
        qT_s = persist.tile([128, 2, 2, S], BF16)   # [p, head, hd-chunk, tok]
        kT_s = persist.tile([128, 2, S], BF16)      # [p, hd-chunk, tok]
        v_s = persist.tile([128, TM, 256], BF16)    # [p(tok), tok-tile, hd]

        pending = []   # (qb, agout) collectives in flight
        staged = []    # (qb, ag_s) gathered results staged in SBUF
        xt_tiles = {}

        def load_xt(tc4):
            t = xtp.tile([128, KT, QB], BF16, tag="xt")
            for g0 in range(0, KT, 5):
                nc.sync.dma_start(
                    t[:, g0:g0 + 5, :],
                    xT_r[:, g0:g0 + 5, tc4 * QB:(tc4 + 1) * QB],
                )
            xt_tiles[tc4] = t

        def rms_rope(ps_pair, nw_s, dst, dst_h, t0):
            """ps_pair: two [128, QB] f32 PSUM chunks of one head's ^T proj.
            Normalize (RMS over the 256 partition dims), scale by per-dim
            weight, apply RoPE, write bf16 into dst[:, (dst_h,) d, t0:t0+QB].
            All row-vectors are kept 128-partition-broadcast (via the ones
            matmul) so every DVE/ACT op runs full-width.
            """
            sqs = []
            for d in range(2):
                sq = scr.tile([128, QB], BF16, tag="sq%d" % d)
                nc.scalar.activation(sq[:], ps_pair[d][:], AF.Square)
                sqs.append(sq)
            ss = psR.tile([128, QB], F32, tag="r")
            nc.tensor.matmul(ss[:], onesm[:], sqs[0][:], start=True, stop=False)
            nc.tensor.matmul(ss[:], onesm[:], sqs[1][:], start=False, stop=True)
            sr = scr.tile([128, QB], F32, tag="sr")
            nc.scalar.activation(sr[:], ss[:], AF.Sqrt, scale=1.0 / HD)
            rr = scr.tile([128, QB], F32, tag="rr")
            nc.vector.reciprocal_approx_fast(rr[:], sr[:])
            tqs = []
            for d in range(2):
                tq = scr.tile([128, QB], BF16, tag="tq%d" % d)
                nc.vector.scalar_tensor_tensor(
                    tq[:], ps_pair[d][:], nw_s[:, d:d + 1], rr[:],
                    ALU.mult, ALU.mult,
                )
                tqs.append(tq)
            for d in range(2):
                a = scr.tile([128, QB], BF16, tag="ra")
                b = scr.tile([128, QB], BF16, tag="rb")
                nc.vector.tensor_mul(a[:], tqs[d][:], cos_s[:, d, t0:t0 + QB])
                nc.vector.tensor_mul(b[:], tqs[1 - d][:], rsin_s[:, d, t0:t0 + QB])
                if dst_h is None:
                    dslice = dst[:, d, t0:t0 + QB]
                else:
                    dslice = dst[:, dst_h, d, t0:t0 + QB]
                nc.vector.tensor_add(dslice, a[:], b[:])

        def proj_chunk(tc4):
            xs = xt_tiles.pop(tc4)[:, :, :]
            t0 = tc4 * QB
            # K^T for this token chunk
            kps = []
            for d in range(2):
                pk = psB.tile([128, QB], F32, tag="b")
                for kt in range(KT):
                    nc.tensor.matmul(
                        pk[:], wk_s[:, kt, d * 128:(d + 1) * 128],
                        xs[:, kt, :], start=(kt == 0), stop=(kt == KT - 1),
                    )
                kps.append(pk)
            rms_rope(kps, kn_s, kT_s, None, t0)
            # Q^T per head
            for h in range(2):
                qps = []
                for d in range(2):
                    pq = psB.tile([128, QB], F32, tag="b")
                    c = 2 * h + d
                    for kt in range(KT):
                        nc.tensor.matmul(
                            pq[:], wq_s[:, kt, c * 128:(c + 1) * 128],
                            xs[:, kt, :], start=(kt == 0), stop=(kt == KT - 1),
                        )
                    qps.append(pq)
                rms_rope(qps, qn_s, qT_s, h, t0)
            # V (natural layout) for the 4 token tiles in this chunk
            for mm in range(4):
                m = (t0 // 128) + mm
                pv = psB.tile([128, QB], F32, tag="b")
                for kt in range(KT):
                    nc.tensor.matmul(
                        pv[:, 0:256], xs[:, kt, mm * 128:(mm + 1) * 128],
                        wv_s[:, kt, :], start=(kt == 0), stop=(kt == KT - 1),
                    )
                nc.vector.tensor_copy(v_s[:, m, :], pv[:, 0:256])

        def att_qb(rep, qb):
            q0 = qb * QB
            kt_lo = max(0, (q0 - WIN) // 128)
            kt_hi = (q0 + QB - 1) // 128
            att = attp.tile([128, 4, QB], BF16, tag="att")
            for h in range(2):
                o_ps = psA.tile([128, 2, QB], F32, tag="acc")
                se_ps = psR.tile([128, QB], F32, tag="r")
                kts = list(range(kt_lo, kt_hi + 1))
                # valid q-column range per k-tile: edge tiles are narrowed
                # (outside columns are fully masked -> skip their compute)
                rng = {}
                for kt in kts:
                    d_off = q0 - kt * 128
                    lo_q = max(0, -d_off)
                    hi_q = min(QB, WIN + 128 - d_off) if d_off >= 640 else QB
                    rng[kt] = (lo_q, hi_q)
                # PV accumulation order: a full-width tile first (so every
                # psum column's first write has start=True), then the rest.
                full0 = next(kt for kt in kts if rng[kt] == (0, QB))
                pv_order = [full0] + [kt for kt in kts if kt != full0]
                epipe = {}
                emitted = [0]
                se_emitted = [0]

                def emit_pv(kt):
                    first, last = kt == pv_order[0], kt == pv_order[-1]
                    lo_q, hi_q = rng[kt]
                    e = epipe.pop(kt)
                    for d in range(2):
                        nc.tensor.matmul(
                            o_ps[:, d, lo_q:hi_q],
                            v_s[:, kt, d * 128:(d + 1) * 128],
                            e[:, lo_q:hi_q], start=first, stop=last,
                        )

                def drain_se(upto):
                    while se_emitted[0] < len(pv_order):
                        kt = pv_order[se_emitted[0]]
                        if kt not in epipe or kts.index(kt) > upto:
                            break
                        lo_q, hi_q = rng[kt]
                        nc.tensor.matmul(
                            se_ps[:, lo_q:hi_q], onesm[:], epipe[kt][:, lo_q:hi_q],
                            start=kt == pv_order[0], stop=kt == pv_order[-1],
                        )
                        se_emitted[0] += 1

                def drain_pv(upto):
                    # emit pending PVs in pv_order whose exp tile exists
                    # and whose scores stage is >= 2 iterations old
                    while emitted[0] < len(pv_order):
                        kt = pv_order[emitted[0]]
                        if kt not in epipe or kts.index(kt) > upto:
                            break
                        emit_pv(kt)
                        emitted[0] += 1

                for i, kt in enumerate(kts):
                    k0 = kt * 128
                    d_off = q0 - k0
                    lo_q, hi_q = rng[kt]
                    s_ps = psB.tile([128, QB], F32, tag="b")
                    for d in range(2):
                        nc.tensor.matmul(
                            s_ps[:, lo_q:hi_q], kT_s[:, d, k0:k0 + 128],
                            qT_s[:, h, d, q0 + lo_q:q0 + hi_q],
                            start=(d == 0), stop=(d == 1),
                        )
                    e_s = esb.tile([128, QB], BF16, tag="e")
                    nc.scalar.activation(e_s[:, lo_q:hi_q], s_ps[:, lo_q:hi_q],
                                         AF.Exp)
                    if not (128 <= d_off <= 512):
                        nc.vector.tensor_mul(
                            e_s[:, lo_q:hi_q], e_s[:, lo_q:hi_q],
                            mask_s[:, 384 + d_off + lo_q:384 + d_off + hi_q],
                        )
                    epipe[kt] = e_s
                    drain_se(i - 1)
                    drain_pv(i - 2)
                drain_se(len(kts) - 1)
                drain_pv(len(kts) - 1)
                rc = scr.tile([128, QB], F32, tag="rc")
                nc.vector.reciprocal_approx_fast(rc[:], se_ps[:])
                for d in range(2):
                    nc.vector.tensor_mul(att[:, 2 * h + d, :], o_ps[:, d, :],
                                         rc[:])
            # AllGather this block's attention outputs across the batch
            # group; consumed by o-proj one phase later.
            agin = dram.tile([512, QB], BF16, tag=f"agin{rep}_{qb}")
            agout = dram.tile([NH * HD, QB], BF16, tag=f"agout{rep}_{qb}")
            agin_r = agin.rearrange("(c p) w -> c p w", p=128)
            for c in range(4):
                nc.sync.dma_start(agin_r[c], att[:, c, :])
            if stub_collective:
                agout_r = agout.rearrange("(r c p) w -> r c p w", p=128, c=4)
                for rr_i in range(4):
                    for cc in range(4):
                        nc.sync.dma_start(agout_r[rr_i, cc], att[:, cc, :])
            else:
                nc.gpsimd.collective_compute(
                    "AllGather",
                    mybir.AluOpType.bypass,
                    replica_groups=REPLICA_GROUPS,
                    ins=[agin[:]],
                    outs=[agout[:]],
                )
            pending.append((qb, agout))

        def stage_one():
            qb, agout = pending.pop(0)
            ag_s = agp.tile([128, TM, QB], BF16, tag="ag")
            agout_r2 = agout.rearrange("(t p) w -> p t w", p=128)
            for mm in range(4):
                nc.sync.dma_start(
                    ag_s[:, :, mm * 128:(mm + 1) * 128],
                    agout_r2[:, :, mm * 128:(mm + 1) * 128],
                )
            staged.append((qb, ag_s))

        def oproj_one():
            qb, ag_s = staged.pop(0)
            for mm in range(4):
                m = qb * 4 + mm
                po = psA.tile([128, NHID_LOC], F32, tag="acc")
                for kt2 in range(TM):
                    lhs = ag_s[:, kt2, mm * 128:(mm + 1) * 128]
                    nc.tensor.matmul(
                        po[:, 0:512], lhs, wo_s[:, kt2, 0:512],
                        start=(kt2 == 0), stop=(kt2 == TM - 1),
                    )
                    nc.tensor.matmul(
                        po[:, 512:NHID_LOC], lhs, wo_s[:, kt2, 512:NHID_LOC],
                        start=(kt2 == 0), stop=(kt2 == TM - 1),
                    )
                ot = outp.tile([128, NHID_LOC], BF16, tag="ot")
                nc.vector.tensor_copy(ot[:], po[:])
                nc.sync.dma_start(out[m * 128:(m + 1) * 128, :], ot[:])

        for rep in range(reps):
            if rep == 0:
                t0c = xtp.tile([128, KT, QB], BF16, tag="xt")
                for a, b in ((0, 2), (2, 5), (5, 10), (10, 15), (15, 20)):
                    nc.sync.dma_start(t0c[:, a:b, :], xT_r[:, a:b, 0:QB])
                    nc.sync.dma_start(wq_s[:, a:b, :], wq_r[:, a:b, :])
                    nc.sync.dma_start(wk_s[:, a:b, :], wk_r[:, a:b, :])
                    nc.sync.dma_start(wv_s[:, a:b, :], wv_r[:, a:b, :])
                xt_tiles[0] = t0c
                nc.sync.dma_start(qn_s[:], qnw_r[:])
                nc.sync.dma_start(kn_s[:], knw_r[:])
                nc.sync.dma_start(cos_s[:], cosT_r[:])
                nc.sync.dma_start(rsin_s[:], rsinT_r[:])
                nc.sync.dma_start(mask_s[:], maskb[:])
                load_xt(1)
                nc.sync.dma_start(wo_s[:], wo_r[:])
            # o-proj drains of the previous rep's last two blocks are
            # interleaved with this rep's first projection phases, so each
            # AllGather (and its SBUF staging DMA) has a full compute phase
            # of runway before the PE touches its data.
            if pending:
                stage_one()
            proj_chunk(0)
            if staged:
                oproj_one()
            if pending:
                stage_one()
            proj_chunk(1)
            if staged:
                oproj_one()
            att_qb(rep, 0)
            load_xt(2)
            att_qb(rep, 1)
            load_xt(3)
            stage_one()
            proj_chunk(2)
            oproj_one()
            stage_one()
            proj_chunk(3)
            oproj_one()
            att_qb(rep, 2)
            if rep + 1 < reps:
                load_xt(0)
            att_qb(rep, 3)
            if rep + 1 < reps:
                load_xt(1)
        while pending:
            stage_one()
            oproj_one()

    nc.compile()
    return nc


_NC = {}


def _build(reps=1):
    if reps not in _NC:
        _NC[reps] = _emit(
            bacc.Bacc("TRN2", target_bir_lowering=False, debug=False,
                      num_devices=NCORES),
            reps=reps,
        )
    return _NC[reps]


def _host_prep(hidden_states, cos, sin, wq, wk, wv, wo, q_norm_w, k_norm_w):
    """Build the 8 per-core input maps (numpy, bf16 where device expects bf16)."""
    f32 = np.float32
    qn = ((1.0 + q_norm_w.astype(f32)) * SCALE).reshape(HD, 1)
    kn = (1.0 + k_norm_w.astype(f32)).reshape(HD, 1)
    # rsin: [-sin_firsthalf, +sin_secondhalf] so rope = q*cos + q[swap]*rsin
    # mask band: maskb[kk, y] = 1 iff 0 <= (y-384) - kk < WIN
    kk = np.arange(128)[:, None]
    y = np.arange(1920)[None, :]
    maskb = ((y - 384 - kk >= 0) & (y - 384 - kk < WIN)).astype(NPBF16)

    in_maps = []
    for c in range(NCORES):
        b, g = divmod(c, 4)
        sin_b = sin[b].astype(f32)
        rsin = np.concatenate([-sin_b[:, :128], sin_b[:, 128:]], axis=1)
        in_maps.append({
            "xT": np.ascontiguousarray(hidden_states[b].T).astype(NPBF16),
            "wq": np.ascontiguousarray(
                wq[:, 2 * g * HD:(2 * g + 2) * HD]).astype(NPBF16),
            "wk": np.ascontiguousarray(wk[:, g * HD:(g + 1) * HD]).astype(NPBF16),
            "wv": np.ascontiguousarray(wv[:, g * HD:(g + 1) * HD]).astype(NPBF16),
            "wo": np.ascontiguousarray(
                wo[:, g * NHID_LOC:(g + 1) * NHID_LOC]).astype(NPBF16),
            "cosT": np.ascontiguousarray(cos[b].T).astype(NPBF16),
            "rsinT": np.ascontiguousarray(rsin.T).astype(NPBF16),
            "qnw": qn,
            "knw": kn,
            "maskb": maskb,
        })
    return in_maps


class _Runner:
    """Compile the Bass module to a reusable 8-device PJRT executable
    (mirrors bass2jax.run_bass_via_pjrt but keeps the jitted fn for
    repeated steady-state invocation)."""

    def __init__(self, nc):
        import jax
        from jax.sharding import Mesh, PartitionSpec
        try:
            from jax import shard_map as _sm
            shard_map = _sm.shard_map if hasattr(_sm, "shard_map") else _sm
        except Exception:
            from jax.experimental.shard_map import shard_map
        from concourse import bass2jax
        from concourse.bass2jax import _bass_exec_p

        bass2jax.install_neuronx_cc_hook()
        self.jax = jax
        self.nc = nc
        part_name = (nc.partition_id_tensor.name
                     if nc.partition_id_tensor else None)
        in_names, out_names, out_avals = [], [], []
        for alloc in nc.m.functions[0].allocations:
            if not isinstance(alloc, mybir.MemoryLocationSet):
                continue
            name = alloc.memorylocations[0].name
            if alloc.kind == "ExternalInput":
                if name != part_name:
                    in_names.append(name)
            elif alloc.kind == "ExternalOutput":
                out_names.append(name)
                out_avals.append(jax.core.ShapedArray(
                    tuple(alloc.tensor_shape), mybir.dt.np(alloc.dtype)))
        self.in_names, self.out_names, self.out_avals = in_names, out_names, out_avals
        all_names = list(in_names) + list(out_names)
        if part_name is not None:
            all_names.append(part_name)

        def _body(*args):
            operands = list(args)
            if part_name is not None:
                operands.append(bass2jax.partition_id_tensor())
            outs = _bass_exec_p.bind(
                *operands,
                out_avals=tuple(out_avals),
                in_names=tuple(all_names),
                out_names=tuple(out_names),
                lowering_input_output_aliases=(),
                sim_require_finite=True,
                sim_require_nnan=True,
                nc=nc,
            )
            return tuple(outs)

        devices = jax.devices()[:NCORES]
        self.mesh = Mesh(np.asarray(devices), ("core",))
        n_args = len(in_names) + len(out_names)
        self.fn = jax.jit(
            shard_map(
                _body, mesh=self.mesh,
                in_specs=(PartitionSpec("core"),) * n_args,
                out_specs=(PartitionSpec("core"),) * len(out_names),
                check_vma=False,
            ),
            keep_unused=True,
        )
        self.sharding = jax.sharding.NamedSharding(
            self.mesh, PartitionSpec("core"))
        self.zeros = [
            jax.device_put(
                np.zeros((NCORES * a.shape[0], *a.shape[1:]), a.dtype),
                self.sharding)
            for a in out_avals
        ]

    def put(self, in_maps):
        concat = [
            np.concatenate([np.asarray(in_maps[c][n]) for c in range(NCORES)],
                           axis=0)
            for n in self.in_names
        ]
        return [self.jax.device_put(a, self.sharding) for a in concat]

    def run(self, in_dev):
        outs = self.fn(*in_dev, *self.zeros)
        return [o.block_until_ready() for o in outs]

    def results(self, outs):
        per_core = []
        for c in range(NCORES):
            m = {}
            for i, n in enumerate(self.out_names):
                a = self.out_avals[i]
                m[n] = np.asarray(outs[i]).reshape(NCORES, *a.shape)[c]
            per_core.append(m)
        return per_core


_RUNNER = None


def _get_runner():
    global _RUNNER
    if _RUNNER is None:
        _RUNNER = _Runner(_build())
    return _RUNNER


def kernel(hidden_states, cos, sin, wq, wk, wv, wo, q_norm_w, k_norm_w):
    global _RUNNER
    in_maps = _host_prep(hidden_states, cos, sin, wq, wk, wv, wo,
                         q_norm_w, k_norm_w)
    # The axon tunnel fails transiently (~1/3 of runs: mesh desync / exec-unit
    # unrecoverable). Retry the dispatch; on repeat failure rebuild the runner.
    last = None
    for attempt in range(4):
        try:
            r = _get_runner()
            res = r.results(r.run(r.put(in_maps)))
            break
        except Exception as e:  # transient axon/NRT dispatch failures
            last = e
            _RUNNER = None
    else:
        raise last
    out = np.empty((B, S, HID), np.float32)
    for b in range(B):
        out[b] = np.concatenate(
            [res[b * 4 + g]["out"].astype(np.float32) for g in range(4)],
            axis=1,
        )
    return out
